# revision 24
# baseline (speedup 1.0000x reference)
"""Trainium2 Bass kernel for nn_Attention_41704132444382.

Masked-linear QKV projection + 16-head attention + masked-linear output
projection. Sharding: batch x head-quad — core c handles batch c//4 and
heads (c%4)*4..(c%4)*4+3. Host sums the 4 per-batch partial outputs and
adds the gated bias.

Per-core pipeline (PSUM accumulates fp32):
  - Mantissa-critical operands in fp16 (x, all weights, Q^T/K^T, attnT):
    fp16 matmuls run 1 cyc/col like bf16 but carry 8x finer mantissa
    (bf16 x alone costs 1e-2 rel err; f32r runs 2 passes on HW).
    Exponent-critical tensors in bf16/f32: exp(scores) reaches ~1e26 and
    1/denominator ~1e-26, far outside fp16 range — e and V are bf16, the
    reciprocal path bf16/f32.
  - Q^T/K^T [2*64, 2048] per head-pair via weight-stationary matmuls;
    the four t-half-0 chains run kt-interleaved so each landing x chunk
    is consumed immediately (bootstrap).
  - V produced directly as [t, dv] tiles (x tile stationary) — no
    transposes. A ones column at slot 64 of each [128, 65] V tile makes
    the PV matmul emit the softmax denominator on PSUM partition 64.
  - S^T [j, i] per pair; exp on ScalarE from PSUM ([128,1024] per pair,
    double-buffered s tiles), scale 1/32 folded in, no max subtraction
    (fp32/bf16 exponent absorbs the range).
  - Normalization: PE transpose gather -> DVE reciprocal -> PE ones
    broadcast -> fused multiply into fp16 attnT.
  - Out-projection: K=128 chains over the two head-pair attnT tiles;
    bf16 partials summed on host in fp32.

Engine queues are in-order FIFOs fixed at compile time, so emission
order IS the schedule. All projection chains / normalization / out-
projection work that must overlap the ScalarE-bound attention loop is
queued as small deferred items and drained one-per-jt between attention
groups — never as block-boundary lumps (those stall the ACT queue
behind a head-of-line PE batch).
"""

import sys
from collections import deque

import numpy as np

sys.path.insert(0, "/opt/trn_rl_repo")

import concourse.bass as bass
import concourse.mybir as mybir
from concourse import bacc
from concourse.masks import make_identity
from concourse.tile import TileContext

DIM = 1024
HEADS = 16
B = 2
N = 2048  # tokens per batch = tokens per core
NCORES = 8
HPC = 4  # heads per core
DV = HPC * 64  # 256 head-dims per core
SCALE = DIM ** (-0.5)  # 1/32

F32 = mybir.dt.float32
F32R = mybir.dt.float32r
FP16 = mybir.dt.float16
BF16 = mybir.dt.bfloat16

NJT = N // 128  # 16 key tiles
NIB = 4  # query blocks of 512
IBW = 512  # i-block width


def build_nc():
    nc = bacc.Bacc("TRN2", target_bir_lowering=True)
    xT_d = nc.declare_dram_parameter("xT", [DIM, N], FP16, isOutput=False)
    wqkT_d = nc.declare_dram_parameter("wqkT", [DIM, 2 * DV], FP16, isOutput=False)
    wvT_d = nc.declare_dram_parameter("wvT", [DIM, DV], FP16, isOutput=False)
    woT_d = nc.declare_dram_parameter("woT", [DV, DIM], FP16, isOutput=False)
    out_d = nc.declare_dram_parameter("out", [N, DIM], BF16, isOutput=True)

    mult = mybir.AluOpType.mult
    Exp = mybir.ActivationFunctionType.Exp

    with TileContext(nc) as tc:
        with tc.tile_pool(name="persist", bufs=1) as pp:
            wqk = pp.tile([128, 8 * 512], FP16)  # [k-part, (kt, qk-col)]
            wv = pp.tile([128, 8 * 256], FP16)  # [k-part, (kt, dv)]
            wo01 = pp.tile([128, 1024], FP16)  # [dv h0|h1, o]
            wo23 = pp.tile([128, 1024], FP16)
            xt = [pp.tile([128, N], FP16, name=f"xt{k}") for k in range(8)]
            qT = [pp.tile([128, N], FP16, name=f"qT{p}") for p in range(2)]  # pair p
            kT = [pp.tile([128, N], FP16, name=f"kT{p}") for p in range(2)]
            v_sb = pp.tile([128, NJT * HPC * 65], BF16)  # [t-part, (jt, h, dv|1)]
            ident = pp.tile([128, 128], F32)
            onesb = pp.tile([1, 64], BF16)

            # ---------- input DMAs ----------
            # big consolidated transfers (descriptor-gen on the queue costs
            # ~650ns per dma_start regardless of size): wqk first on the sync
            # ring, then x in t-half chunks in consumption order; wv/wo ride
            # the scalar ring in parallel
            nc.sync.dma_start(
                wqk[:].rearrange("p (kt o) -> p kt o", kt=8),
                wqkT_d[:].rearrange("(kt p) o -> p kt o", p=128),
            )
            nc.scalar.dma_start(
                wv[:].rearrange("p (kt o) -> p kt o", kt=8),
                wvT_d[:].rearrange("(kt p) o -> p kt o", p=128),
            )
            nc.scalar.dma_start(wo01[:], woT_d[0:128, :])
            nc.scalar.dma_start(wo23[:], woT_d[128:256, :])
            for half in range(2):
                for k in range(8):
                    nc.sync.dma_start(
                        xt[k][:, half * 1024 : (half + 1) * 1024],
                        xT_d[k * 128 : (k + 1) * 128, half * 1024 : (half + 1) * 1024],
                    )

            make_identity(nc, ident[:])
            ones_f = pp.tile([128, 64], F32)
            nc.vector.memset(ones_f[:], 1.0)
            nc.vector.tensor_copy(onesb[:], ones_f[0:1, :])
            # ones column at slot 64 of each 65-wide V block (V writes 0..63)
            nc.vector.tensor_copy(
                v_sb[:].rearrange("p (b c) -> p b c", c=65)[:, :, 64:65],
                ones_f[:, 0 : NJT * HPC].rearrange("p (b c) -> p b c", c=1),
            )

            with (
                tc.tile_pool(name="spool", bufs=2, space="PSUM") as sp,
                tc.tile_pool(name="pvpool", bufs=2, space="PSUM") as pvp,
                tc.tile_pool(name="oppool", bufs=2, space="PSUM") as opp,
                tc.tile_pool(name="epool", bufs=6) as ep,
                tc.tile_pool(name="evac", bufs=2) as vp,
                tc.tile_pool(name="unpool", bufs=8) as up,
                tc.tile_pool(name="obpool", bufs=4) as obp,
            ):
                pending = deque()  # deferred emission items (zero-arg closures)

                def drain(n):
                    for _ in range(n):
                        if pending:
                            pending.popleft()()

                def emit_qk(o, th):
                    # o: 0/1 -> q pair0/pair1, 2/3 -> k pair0/pair1
                    ps = opp.tile([128, 512], F32, tag="op", name=f"qk{o}_{th}")
                    for kt in range(8):
                        nc.tensor.matmul(
                            ps[:],
                            wqk[:, kt * 512 + o * 128 : kt * 512 + (o + 1) * 128],
                            xt[kt][:, th * 512 : (th + 1) * 512],
                            start=(kt == 0),
                            stop=(kt == 7),
                        )
                    dest = (qT + kT)[o]
                    nc.vector.tensor_copy(dest[:, th * 512 : (th + 1) * 512], ps[:])

                def emit_v(tt):
                    ps = opp.tile([128, 512], F32, tag="op", name=f"vps{tt}")
                    for kt in range(8):
                        nc.tensor.matmul(
                            ps[:, 0:256],
                            xt[kt][:, tt * 128 : (tt + 1) * 128],
                            wv[:, kt * 256 : (kt + 1) * 256],
                            start=(kt == 0),
                            stop=(kt == 7),
                        )
                    nc.vector.tensor_copy(
                        v_sb[:, tt * 4 * 65 : (tt + 1) * 4 * 65].rearrange(
                            "p (h c) -> p h c", c=65
                        )[:, :, 0:64],
                        ps[:, 0:256].rearrange("p (h c) -> p h c", c=64),
                    )

                def norm_items(ib, pair, cs_sb, unorm, attnT_blk):
                    # -> 3 deferred items computing attnT for this head-pair
                    rcp = vp.tile([128, 8], F32, tag=f"rcp{pair}", name=f"rcp{ib}_{pair}")
                    at = vp.tile([128, 512], FP16, tag=f"at{pair}", name=f"at{ib}_{pair}")
                    attnT_blk.append(at)

                    def i_recip():
                        pt = opp.tile([128, 512], F32, tag="op", name=f"pt{ib}_{pair}")
                        for i in range(8):  # i = hh*4 + c
                            idx = pair * 8 + i
                            nc.tensor.transpose(
                                pt[:, i : i + 1],
                                cs_sb[0:1, idx * 128 : (idx + 1) * 128],
                                ident[0:1, 0:1],
                            )
                        rr = vp.tile(
                            [128, 8], F32, tag=f"rr{pair}", name=f"rr{ib}_{pair}"
                        )
                        nc.vector.tensor_copy(rr[:], pt[:, 0:8])
                        nc.vector.reciprocal(rcp[:], rr[:])

                    def mk_head(hh):
                        h = pair * 2 + hh

                        def i_pr():
                            pr = opp.tile(
                                [128, 512], F32, tag="op", name=f"pr{ib}_{h}"
                            )
                            for c in range(4):
                                nc.tensor.transpose(
                                    pr[0:1, c * 128 : (c + 1) * 128],
                                    rcp[:, hh * 4 + c : hh * 4 + c + 1],
                                    ident[:],
                                )
                            r2 = vp.tile(
                                [1, 512], BF16, tag=f"r2_{h}", name=f"r2_{ib}_{h}"
                            )
                            nc.vector.tensor_copy(r2[0:1, :], pr[0:1, 0:512])
                            rb = opp.tile(
                                [128, 512], F32, tag="op", name=f"rb{ib}_{h}"
                            )
                            nc.tensor.matmul(
                                rb[0:64, :], onesb[:], r2[:], start=True, stop=True
                            )
                            nc.vector.tensor_tensor(
                                at[hh * 64 : (hh + 1) * 64, :],
                                unorm[h][:],
                                rb[0:64, :],
                                mult,
                            )

                        return i_pr

                    return [i_recip, mk_head(0), mk_head(1)]

                def outproj_items(ib, attnT, use_scalar=False):
                    items = []
                    for tt in range(4):
                        for oh in range(2):
                            def i_po(tt=tt, oh=oh):
                                po = opp.tile(
                                    [128, 512], F32, tag="op", name=f"po{ib}_{tt}_{oh}"
                                )
                                nc.tensor.matmul(
                                    po[:],
                                    attnT[0][:, tt * 128 : (tt + 1) * 128],
                                    wo01[:, oh * 512 : (oh + 1) * 512],
                                    start=True,
                                    stop=False,
                                )
                                nc.tensor.matmul(
                                    po[:],
                                    attnT[1][:, tt * 128 : (tt + 1) * 128],
                                    wo23[:, oh * 512 : (oh + 1) * 512],
                                    start=False,
                                    stop=True,
                                )
                                ob = obp.tile(
                                    [128, 512], BF16, tag="ob", name=f"ob{ib}_{tt}_{oh}"
                                )
                                if use_scalar:
                                    nc.scalar.copy(ob[:], po[:])
                                else:
                                    nc.vector.tensor_copy(ob[:], po[:])
                                nc.sync.dma_start(
                                    out_d[
                                        ib * 512 + tt * 128 : ib * 512 + (tt + 1) * 128,
                                        oh * 512 : (oh + 1) * 512,
                                    ],
                                    ob[:],
                                )

                            items.append(i_po)
                    return items

                # ---------- phase 1 bootstrap: K/Q for t-half 0 ----------
                # four chains interleaved kt-wise so each x chunk is consumed
                # the moment it lands (single pass over x half 0)
                boot_o = [2, 3, 0, 1]
                boot_ps = [
                    sp.tile([128, 512], F32, tag="s", name=f"boot{i}")
                    for i in range(2)
                ] + [
                    opp.tile([128, 512], F32, tag="op", name=f"boot{i}")
                    for i in range(2)
                ]
                for kt in range(8):
                    for idx, o in enumerate(boot_o):
                        nc.tensor.matmul(
                            boot_ps[idx][:],
                            wqk[:, kt * 512 + o * 128 : kt * 512 + (o + 1) * 128],
                            xt[kt][:, 0:512],
                            start=(kt == 0),
                            stop=(kt == 7),
                        )
                for idx, o in enumerate(boot_o):
                    nc.vector.tensor_copy((qT + kT)[o][:, 0:512], boot_ps[idx][:])
                # remaining projection chains drain inside block 0, ordered so
                # each V(tt)/K(th) chain precedes its first consumer jt
                ev = lambda tt: (lambda: emit_v(tt))
                ek = lambda o, th: (lambda: emit_qk(o, th))
                pending.extend(
                    [ev(t) for t in range(4)]
                    + [ek(2, 1), ek(3, 1)]
                    + [ev(t) for t in range(4, 8)]
                    + [ek(2, 2), ek(3, 2)]
                    + [ev(t) for t in range(8, 12)]
                    + [ek(2, 3), ek(3, 3)]
                    + [ev(t) for t in range(12, 16)]
                )
                # per-jt drain counts for block 0 pass A (deadline-ordered):
                # V(tt) must drain by jt=tt, K(th) by jt=4*th
                b0_drains = [1, 1, 1, 1, 3, 1, 1, 1, 3, 1, 1, 1, 3, 1, 1, 2]

                # ---------- attention blocks ----------
                prev = None
                for ib in range(NIB):
                    if prev is not None:
                        pending.extend(outproj_items(*prev))
                    i0 = ib * IBW
                    cs_sb = vp.tile([1, 2048], F32, tag="cs", name=f"cs{ib}")
                    unorm = [
                        up.tile([64, 512], F32, tag="un", name=f"un{ib}_{h}")
                        for h in range(4)
                    ]
                    attnT_blk = []
                    for pair in range(2):
                        pv = [
                            pvp.tile([65, 512], F32, tag="pv", name=f"pv{ib}_{pair}_{hh}")
                            for hh in range(2)
                        ]
                        for jt in range(NJT):
                            drain(b0_drains[jt] if (ib == 0 and pair == 0) else 1)
                            s = sp.tile([128, 1024], F32, tag="s", name=f"s{ib}_{pair}_{jt}")
                            for hh in range(2):
                                nc.tensor.matmul(
                                    s[:, hh * 512 : (hh + 1) * 512],
                                    kT[pair][
                                        hh * 64 : (hh + 1) * 64,
                                        jt * 128 : (jt + 1) * 128,
                                    ],
                                    qT[pair][hh * 64 : (hh + 1) * 64, i0 : i0 + IBW],
                                    start=True,
                                    stop=True,
                                )
                            e = ep.tile([128, 1024], BF16, tag="e", name=f"e{ib}_{pair}_{jt}")
                            nc.scalar.activation(e[:], s[:], Exp, scale=SCALE)
                            for hh in range(2):
                                h = pair * 2 + hh
                                jv = (jt * 4 + h) * 65
                                nc.tensor.matmul(
                                    pv[hh][:],
                                    v_sb[:, jv : jv + 65],
                                    e[:, hh * 512 : (hh + 1) * 512],
                                    start=(jt == 0),
                                    stop=(jt == NJT - 1),
                                )
                        tail = ib == NIB - 1 and pair == 1  # ScalarE idle then
                        for hh in range(2):
                            h = pair * 2 + hh
                            if tail:
                                nc.scalar.copy(
                                    cs_sb[0:1, h * 512 : (h + 1) * 512],
                                    pv[hh][64:65, :],
                                )
                                nc.scalar.copy(unorm[h][:], pv[hh][0:64, :])
                            else:
                                nc.vector.tensor_copy(
                                    cs_sb[0:1, h * 512 : (h + 1) * 512],
                                    pv[hh][64:65, :],
                                )
                                nc.vector.tensor_copy(unorm[h][:], pv[hh][0:64, :])
                        pending.extend(norm_items(ib, pair, cs_sb, unorm, attnT_blk))
                    # next block's Q chains must precede its S matmuls in the
                    # PE queue: put them at the FRONT of the deferred queue
                    if ib + 1 < NIB:
                        pending.appendleft(ek(1, ib + 1))
                        pending.appendleft(ek(0, ib + 1))
                    prev = (ib, attnT_blk)

                drain(len(pending))
                for it in outproj_items(*prev, use_scalar=True):
                    it()

    nc.compile()
    return nc


_NC = None


def _get_nc():
    global _NC
    if _NC is None:
        _NC = build_nc()
    return _NC


def _gate(mask):
    """Exact jax fp32 gate: sigmoid(m) > 0.5 (fp32 logistic rounding)."""
    mask = np.asarray(mask, dtype=np.float32)
    return (np.float32(1.0) / (np.float32(1.0) + np.exp(-mask))) > np.float32(0.5)


def make_in_maps(x, qkv_weight, qkv_weight_mask, out_weight, out_weight_mask):
    x = np.asarray(x, dtype=np.float32)
    wq = np.asarray(qkv_weight, dtype=np.float32) * _gate(qkv_weight_mask)
    wo = np.asarray(out_weight, dtype=np.float32) * _gate(out_weight_mask)

    in_maps = []
    for c in range(NCORES):
        b, g = divmod(c, 4)
        r0 = g * DV
        xT = np.ascontiguousarray(x[b].T.astype(np.float16))
        wqkc = np.concatenate(
            [wq[r0 : r0 + DV], wq[DIM + r0 : DIM + r0 + DV]], axis=0
        )  # [512, 1024] rows = (q h0..h3 | k h0..h3)
        in_maps.append(
            {
                "xT": xT,
                "wqkT": np.ascontiguousarray(wqkc.T.astype(np.float16)),
                "wvT": np.ascontiguousarray(wq[2 * DIM + r0 : 2 * DIM + r0 + DV].T.astype(np.float16)),
                "woT": np.ascontiguousarray(wo[:, r0 : r0 + DV].T.astype(np.float16)),
            }
        )
    return in_maps


LAST_RESULTS = None  # BassKernelResults of the most recent run (for profiling)


def kernel(
    x,
    qkv_weight,
    qkv_weight_mask,
    out_weight,
    out_weight_mask,
    out_bias,
    out_bias_mask,
    _trace=False,
    _tmpdir=None,
):
    global LAST_RESULTS
    from concourse.bass_utils import run_bass_kernel_spmd

    nc = _get_nc()
    in_maps = make_in_maps(x, qkv_weight, qkv_weight_mask, out_weight, out_weight_mask)
    res = run_bass_kernel_spmd(
        nc, in_maps, list(range(NCORES)), trace=_trace, tmpdir=_tmpdir
    )
    LAST_RESULTS = res
    out = np.zeros((B, N, DIM), dtype=np.float32)
    for c, r in enumerate(res.results):
        out[c // 4] += np.asarray(r["out"], dtype=np.float32)
    out_bias = np.asarray(out_bias, dtype=np.float32)
    out += np.where(_gate(out_bias_mask), out_bias, np.float32(0.0))[None, None, :]
    return out
